# revision 8
# baseline (speedup 1.0000x reference)
"""Expert-parallel MoE block (dense path) on 8 Trainium2 NeuronCores.

Reference computation (E=8, C=1024, D_IN=4096, D_OUT=1024, N_TOK=8192):
    expert_out = einsum('eci,eio->eco', expert_input, weight) + bias   # [E,C,D_OUT]
    output     = combine_weights @ expert_out.reshape(E*C, D_OUT)      # [N_TOK,D_OUT]

Sharding (expert-parallel):
  Core e holds expert e: computes X_e = expert_input[e] @ weight[e] + bias[e]
  ([C, D_OUT]); on-device AllGathers assemble X; core e computes its token
  slice of the combine; the host concatenates the 8 row blocks.

Numerics: all matmuls are fp8-e4m3 in DoubleRow mode (2 k-tiles per pass).
The error budget works out because the output is dominated by the bias/mean
structure, which is carried exactly in fp32:
  - Expert GEMM: A in fp8, W*64 in fp8 (scaling keeps W out of e4m3
    subnormals); PSUM fp32; drain computes X' = psum/64 + (bias - mu) on DVE
    and casts to fp8.  mu = colmean(X) is computed EXACTLY on the host from
    the quantized inputs (colsum commutes with the GEMM).
  - Combine: out = CW'@X' + rowsum(CW) (x) colsum(X) / K, with CW' = CW-0.5
    quantized fp8 on the host. Centering both operands halves their rms so
    the fp8 noise lands ~1e-3 relative, and the exact rank-1 correction is a
    per-(tb,ti) DVE tensor_scalar init of the fp32 accumulator.
Measured end-to-end rel err ~1.6e-3.

Performance structure (v2): the PE is issue-limited at the (GPIO-throttled,
13/16) clock, so the kernel is organized to keep it on REAL work end to end:
  - A dummy 16-byte AllGather is triggered as the very first gpsimd
    instruction: the ~55us first-collective rank barrier runs concurrently
    with the input DMA + expert GEMM instead of serializing after it.
  - The expert GEMM sweeps c-tiles; after each c-tile PAIR the fp8 X' tiles
    are evicted and an AllGather chunk (2 c-tiles x 8 ranks, 2MB out) is
    triggered: 4 uniform chunks, each a single kp-aligned DoubleRow pair, so
    every ck/xk tile is one contiguous DMA. Chunks post at ~36/53/70/86us;
    the post-barrier AGs run back-to-back and each lands 10-50us before the
    combine consumes it.
  - The combine starts as soon as (expert GEMM done, AG0+xk0 landed) and
    runs gap-free: no filler matmuls needed, PE does only real work.
"""

import numpy as np

E = 8
C = 1024
D_IN = 4096
D_OUT = 1024
N_TOK = E * C
P = 128

KP1 = D_IN // (2 * P)  # 16 k-tile pairs in the expert GEMM
NCH = 4  # AllGather chunks: c-tile pairs (0,1), (2,3), (4,5), (6,7)

_cached = None


def _build():
    import concourse.bass as bass  # noqa: F401
    import concourse.mybir as mybir
    import concourse.tile as tile
    from concourse import bacc

    F8 = mybir.dt.float8e4
    F32 = mybir.dt.float32
    DR = mybir.MatmulPerfMode.DoubleRow

    nc = bacc.Bacc("TRN2", target_bir_lowering=False, debug=False, num_devices=E)

    at = nc.dram_tensor("at", [D_IN, C], F8, kind="ExternalInput").ap()
    w = nc.dram_tensor("w", [D_IN, D_OUT], F8, kind="ExternalInput").ap()
    badj = nc.dram_tensor("badj", [1, D_OUT], F32, kind="ExternalInput").ap()
    svec = nc.dram_tensor("svec", [1, D_OUT], F32, kind="ExternalInput").ap()
    alpha = nc.dram_tensor("alpha", [C, 1], F32, kind="ExternalInput").ap()
    cwt = nc.dram_tensor("cwt", [N_TOK, C], F8, kind="ExternalInput").ap()
    out = nc.dram_tensor("out", [C, D_OUT], F32, kind="ExternalOutput").ap()

    xh = [nc.dram_tensor(f"xh{b}", [2 * P, D_OUT], F8) for b in range(NCH)]
    xg = [
        nc.dram_tensor(f"xg{b}", [E * 2 * P, D_OUT], F8, addr_space="Shared")
        for b in range(NCH)
    ]

    at4 = at.rearrange("(q kp t p) c -> p q (kp t) c", p=P, t=2, q=4)  # [128,4,8,1024]
    w4 = w.rearrange("(q kp t p) d -> p q (kp t) d", p=P, t=2, q=4)  # [128,4,8,1024]
    cwt4 = cwt.rearrange("(kp t p) c -> p kp t c", p=P, t=2)  # [128, 32, 2, 1024]
    xh3 = [x.rearrange("(ci p) d -> p ci d", p=P) for x in xh]
    # gathered chunk b: rank-major [8 ranks, 2 c-tiles, 128, D_OUT]
    xg4 = [x.rearrange("(j t p) d -> p j t d", p=P, t=2) for x in xg]
    al3 = alpha.rearrange("(g p) o -> p g o", p=P)  # [128, 8, 1]
    out4 = out.rearrange("(tb ti p) d -> p tb ti d", p=P, ti=2)  # [128, 4, 2, 1024]

    rg = [list(range(E))]

    with tile.TileContext(nc) as tc:
        with (
            tc.tile_pool(name="wpool", bufs=1) as wpool,
            tc.tile_pool(name="apool", bufs=1) as apool,
            tc.tile_pool(name="cpool", bufs=1) as cpool,
            tc.tile_pool(name="xepool", bufs=2) as xepool,
            tc.tile_pool(name="tmppool", bufs=2) as tmppool,
            tc.tile_pool(name="ckpool", bufs=28) as ckpool,
            tc.tile_pool(name="xkpool", bufs=16) as xkpool,
            tc.tile_pool(name="accpool", bufs=1) as accpool,
            tc.tile_pool(name="ps", bufs=4, space="PSUM") as pspool,
        ):
            # ---- small fp32 constants on the scalar queue (land ~2us) ----
            bias_sb = cpool.tile([P, D_OUT], F32, tag="badj")
            nc.scalar.dma_start(bias_sb[:], badj.to_broadcast((P, D_OUT)))
            svec_sb = cpool.tile([P, D_OUT], F32, tag="svec")
            nc.scalar.dma_start(svec_sb[:], svec.to_broadcast((P, D_OUT)))
            al_sb = cpool.tile([P, 8], F32, tag="al")
            nc.scalar.dma_start(al_sb[:], al3[:, :, 0])

            # ---- resident fp8 A / W: 8 quarter-sized DMAs (one per 1MB)
            # so the sync queue's ~1us/DMA issue rate never starves the PE
            a_q = []
            w_q = []
            for q in range(4):
                ta = apool.tile([P, 8, C], F8, tag=f"a{q}", name=f"a{q}")
                nc.sync.dma_start(ta[:], at4[:, q, :, :])
                a_q.append(ta)
                tw = wpool.tile([P, 8, D_OUT], F8, tag=f"w{q}", name=f"w{q}")
                nc.sync.dma_start(tw[:], w4[:, q, :, :])
                w_q.append(tw)

            # ---- combine accumulator init (DVE is idle early): exact
            # rank-1 term acc[t, d] = alpha[t] * S[d]
            acc = accpool.tile([P, 4, 2, D_OUT], F32)
            for tb in range(4):
                for ti in range(2):
                    nc.vector.tensor_scalar(
                        acc[:, tb, ti, :],
                        svec_sb[:],
                        al_sb[:, tb * 2 + ti : tb * 2 + ti + 1],
                        None,
                        mybir.AluOpType.mult,
                    )

            ck = {}
            xk = {}

            def load_ck(b):
                # chunk b slot j covers k-tiles (j, 2b) and (j, 2b+1) --
                # one kp-aligned cwt4 pair, single DMA.
                for j in range(E):
                    t = ckpool.tile([P, 2, C], F8, tag="ck", name=f"ck_{b}_{j}")
                    nc.gpsimd.dma_start(t[:], cwt4[:, j * 4 + b, :, :])
                    ck[(b, j)] = t

            def load_xk(b):
                # split across the scalar and sync queues so the tiles land
                # in half the serial-DMA time after the AllGather posts
                for j in range(E):
                    t = xkpool.tile([P, 2, D_OUT], F8, tag="xk", name=f"xk_{b}_{j}")
                    eng = nc.scalar if j % 2 == 0 else nc.sync
                    eng.dma_start(t[:], xg4[b][:, j, :, :])
                    xk[(b, j)] = t

            # ---------------- expert GEMM (fp8 DoubleRow) ----------------
            for ci in range(8):
                ps = pspool.tile([P, 2, 512], F32, tag="ps", name=f"ps_e{ci}")
                for kp in range(KP1):
                    lhsT = a_q[kp // 4][
                        :, (kp % 4) * 2 : (kp % 4) * 2 + 2, ci * 128 : (ci + 1) * 128
                    ]
                    for h in range(2):
                        nc.tensor.matmul(
                            ps[:, h, :],
                            lhsT,
                            w_q[kp // 4][
                                :,
                                (kp % 4) * 2 : (kp % 4) * 2 + 2,
                                h * 512 : (h + 1) * 512,
                            ],
                            start=(kp == 0),
                            stop=(kp == KP1 - 1),
                            perf_mode=DR,
                        )
                # X'_ci = psum/64 + (bias - mu), cast fp8
                xe = xepool.tile([P, D_OUT], F8, tag="xe")
                for h in range(2):
                    sl = slice(h * 512, (h + 1) * 512)
                    tmp = tmppool.tile([P, 512], F32, tag="tmp")
                    nc.vector.tensor_scalar_mul(tmp[:], ps[:, h, :], 0.015625)
                    nc.vector.tensor_tensor(
                        xe[:, sl], tmp[:], bias_sb[:, sl], mybir.AluOpType.add
                    )
                b = ci // 2
                nc.gpsimd.dma_start(xh3[b][:, ci % 2, :], xe[:])
                if ci % 2 == 1:
                    nc.gpsimd.collective_compute(
                        "AllGather",
                        mybir.AluOpType.bypass,
                        replica_groups=rg,
                        ins=[xh[b].ap().opt()],
                        outs=[xg[b].ap().opt()],
                    )
                    load_xk(b)  # scalar/sync queues, gated on the AG

            # ck loads AFTER all evictions + AG triggers on the gpsimd
            # queue: a pool-capacity stall here can no longer delay a
            # trigger.  28 bufs: only the last 4 tiles of chunk 3 ever
            # wait (for chunk-0 frees at ~130us, needed ~190us).
            for b in range(NCH):
                load_ck(b)

            # HAM keep-warm filler: bridge the ~10us between expert-GEMM
            # end and the first AllGather's xk tiles landing, so the
            # combine starts on a warm (8/8) PE clock.
            for g in range(5):
                psf = pspool.tile([P, 2, 512], F32, tag="ps", name=f"ps_f{g}")
                for i in range(8):
                    nc.tensor.matmul(
                        psf[:, i % 2, :],
                        a_q[0][:, 0:2, :128],
                        w_q[0][:, 0:2, (i % 2) * 512 : (i % 2) * 512 + 512],
                        start=(i < 2),
                        stop=(i >= 6),
                        perf_mode=DR,
                    )
                tmpf = tmppool.tile([P, 512], F32, tag="tmp")
                nc.vector.tensor_copy(tmpf[:], psf[:, 0, :])

            # ---------------- combine GEMM (fp8 DoubleRow) ----------------
            for b in range(NCH):
                for tb in range(4):
                    for ti in range(2):
                        pst = pspool.tile(
                            [P, 2, 512], F32, tag="ps", name=f"ps_c{b}_{tb}_{ti}"
                        )
                        for s in range(E):
                            lhsT = ck[(b, s)][
                                :, :, tb * 256 + ti * 128 : tb * 256 + (ti + 1) * 128
                            ]
                            for h in range(2):
                                nc.tensor.matmul(
                                    pst[:, h, :],
                                    lhsT,
                                    xk[(b, s)][:, :, h * 512 : (h + 1) * 512],
                                    start=(s == 0),
                                    stop=(s == E - 1),
                                    perf_mode=DR,
                                )
                        for h in range(2):
                            sl = slice(h * 512, (h + 1) * 512)
                            nc.vector.tensor_tensor(
                                acc[:, tb, ti, sl],
                                pst[:, h, :],
                                acc[:, tb, ti, sl],
                                mybir.AluOpType.add,
                            )
                            if b == NCH - 1:
                                # stream each half out as soon as it's final
                                nc.sync.dma_start(
                                    out4[:, tb, ti, sl], acc[:, tb, ti, sl]
                                )

    nc.compile()
    return nc


def _prep_inputs(expert_input, weight, bias, combine_weights):
    import ml_dtypes

    f8 = ml_dtypes.float8_e4m3
    f32 = np.float32

    def q8(x):
        return np.clip(x, -240.0, 240.0).astype(f8)

    A8 = [q8(expert_input[e]) for e in range(E)]  # [C, D_IN]
    W8 = [q8(64.0 * weight[e]) for e in range(E)]  # [D_IN, D_OUT]
    # exact colsum of the fp8 pipeline's X (colsum commutes with the GEMM)
    S = np.zeros(D_OUT, dtype=np.float64)
    for e in range(E):
        S += (
            A8[e].astype(np.float64).sum(0) @ W8[e].astype(np.float64)
        ) / 64.0 + C * bias[e].reshape(-1).astype(np.float64)
    mu = (S / N_TOK).astype(f32)
    Sf = S.astype(f32)

    in_maps = []
    for e in range(E):
        cw = combine_weights[e * C : (e + 1) * C, :]
        r = cw.astype(np.float64).sum(1)
        in_maps.append(
            {
                "at": np.ascontiguousarray(A8[e].T),
                "w": np.ascontiguousarray(W8[e]),
                "badj": np.ascontiguousarray(
                    (bias[e].reshape(1, D_OUT) - mu[None, :]).astype(f32)
                ),
                "svec": np.ascontiguousarray(Sf[None, :]),
                "alpha": np.ascontiguousarray(
                    (r / N_TOK).astype(f32)[:, None]
                ),
                "cwt": np.ascontiguousarray(q8(cw - 0.5).T),
            }
        )
    return in_maps


def _run(expert_input, weight, bias, combine_weights, trace=False):
    from concourse import bass_utils

    global _cached
    if _cached is None:
        _cached = _build()
    nc = _cached
    in_maps = _prep_inputs(expert_input, weight, bias, combine_weights)
    r = bass_utils.run_bass_kernel_spmd(
        nc, in_maps, core_ids=list(range(E)), trace=trace
    )
    output = np.concatenate([r.results[e]["out"] for e in range(E)], axis=0)
    return output.astype(np.float32, copy=False), r


def kernel(expert_input, weight, bias, combine_weights):
    output, _ = _run(expert_input, weight, bias, combine_weights)
    return output


# revision 9
# speedup vs baseline: 1.0388x; 1.0388x over previous
"""Expert-parallel MoE block (dense path) on 8 Trainium2 NeuronCores.

Reference computation (E=8, C=1024, D_IN=4096, D_OUT=1024, N_TOK=8192):
    expert_out = einsum('eci,eio->eco', expert_input, weight) + bias   # [E,C,D_OUT]
    output     = combine_weights @ expert_out.reshape(E*C, D_OUT)      # [N_TOK,D_OUT]

Sharding (expert-parallel):
  Core e holds expert e: computes X_e = expert_input[e] @ weight[e] + bias[e]
  ([C, D_OUT]); on-device AllGathers assemble X; core e computes its token
  slice of the combine; the host re-assembles the 8 row blocks.

Numerics: all matmuls are fp8-e4m3 in DoubleRow mode (2 k-tiles per pass).
The error budget works out because the output is dominated by the bias/mean
structure, which is carried exactly in fp32:
  - Expert GEMM: A in fp8, W*64 in fp8 (scaling keeps W out of e4m3
    subnormals); PSUM fp32; drain computes X' = psum/64 + (bias - mu) on DVE
    and casts to fp8.  mu = colmean(X) is computed EXACTLY on the host from
    the quantized inputs (colsum commutes with the GEMM).
  - Combine: out = CW'@X' + rowsum(CW) (x) colsum(X) / K, with CW' = CW-0.5
    quantized fp8 on the host. Centering both operands halves their rms so
    the fp8 noise lands ~1e-3 relative, and the exact rank-1 correction is a
    per-(tb,ti) DVE tensor_scalar init of the fp32 accumulator.
Measured end-to-end rel err ~1.6e-3.

Performance structure (v4): the PE is issue-limited at the (GPIO-throttled,
13/16 = 1.95GHz) clock; the kernel keeps it on real work end to end.
  - ALL large tensors are passed from the host in partition-major layout
    ([128, bytes-per-partition] with each partition's data contiguous), so
    every DMA moves 2-16KB contiguous runs instead of the 1KB rows of the
    natural layouts (which measured ~140GB/s and starved the expert GEMM).
    A and W load as 4 quarter DMAs each; each ck chunk is ONE DMA.
  - Expert GEMM sweeps c-tiles; X' c-tiles evict to p-major xh buffers and
    AllGather in chunks [1,1,2,2,2]: the 1-tile chunks trigger the (fixed,
    ~60-80us) first-collective barrier as early as every rank can, and the
    later 2-tile chunks amortize the ~2us inter-collective gap.
  - The combine consumes chunks in arrival order with a SBUF fp32
    accumulator; a short HAM keep-warm filler bridges expert-end -> first
    xk tiles so the clock gate never drops to 4/8.
"""

import numpy as np

E = 8
C = 1024
D_IN = 4096
D_OUT = 1024
N_TOK = E * C
P = 128

KP1 = D_IN // (2 * P)  # 16 k-tile pairs in the expert GEMM
CHUNKS = [[0], [1], [2, 3], [4, 5], [6, 7]]
NCH = len(CHUNKS)
NFILL = 5  # HAM keep-warm groups between expert GEMM and combine

# cwt2 block order: for the cross-rank-paired 1-tile chunks, tile u pairs
# k-tiles (2u*8+b, (2u+1)*8+b); for 2-tile chunks, tile j pairs (j*8+c0,
# j*8+c0+1).  The host lays the 64 k-tiles out in exactly this order so
# each chunk's ck tiles are one contiguous DMA.
def _ck_ktile_order():
    order = []
    for b in (0, 1):
        for u in range(4):
            order += [(2 * u) * 8 + b, (2 * u + 1) * 8 + b]
    for b in (2, 3, 4):
        c0 = CHUNKS[b][0]
        for j in range(E):
            order += [j * 8 + c0, j * 8 + c0 + 1]
    return order


_cached = None


def _build():
    import concourse.bass as bass  # noqa: F401
    import concourse.mybir as mybir
    import concourse.tile as tile
    from concourse import bacc

    F8 = mybir.dt.float8e4
    F32 = mybir.dt.float32
    DR = mybir.MatmulPerfMode.DoubleRow

    nc = bacc.Bacc("TRN2", target_bir_lowering=False, debug=False, num_devices=E)

    # partition-major inputs: [128, contiguous bytes per partition]
    at2 = nc.dram_tensor("at2", [P, KP1 * 2 * C], F8, kind="ExternalInput").ap()
    w2 = nc.dram_tensor("w2", [P, KP1 * 2 * D_OUT], F8, kind="ExternalInput").ap()
    badj = nc.dram_tensor("badj", [P, D_OUT], F32, kind="ExternalInput").ap()
    svec = nc.dram_tensor("svec", [P, D_OUT], F32, kind="ExternalInput").ap()
    alpha = nc.dram_tensor("alpha", [P, 8], F32, kind="ExternalInput").ap()
    cwt2 = nc.dram_tensor("cwt2", [P, 64 * C], F8, kind="ExternalInput").ap()
    out = nc.dram_tensor("out", [P, 8 * D_OUT], F32, kind="ExternalOutput").ap()

    # AllGather staging, p-major: xh[b] rank-local [128, len*1024]; the AG
    # concatenates rank blocks so xg[b] rows [j*128:(j+1)*128] are rank j.
    xh = [
        nc.dram_tensor(f"xh{b}", [P, len(ch) * D_OUT], F8)
        for b, ch in enumerate(CHUNKS)
    ]
    xg = [
        nc.dram_tensor(
            f"xg{b}", [E * P, len(ch) * D_OUT], F8, addr_space="Shared"
        )
        for b, ch in enumerate(CHUNKS)
    ]

    atR = at2.rearrange("p (kp t c) -> p kp t c", t=2, c=C)  # [128,16,2,1024]
    wR = w2.rearrange("p (kp t d) -> p kp t d", t=2, d=D_OUT)
    cwtR = cwt2.rearrange("p (blk t c) -> p blk t c", t=2, c=C)  # [128,32,2,1024]
    xgA = [xg[b].rearrange("(j p) c -> p j c", p=P) for b in range(2)]
    xgC = [
        xg[b].rearrange("(j p) (t c) -> p j t c", p=P, t=2) for b in range(2, NCH)
    ]
    out4 = out.rearrange("p (tb ti d) -> p tb ti d", ti=2, d=D_OUT)

    rg = [list(range(E))]

    with tile.TileContext(nc) as tc:
        with (
            tc.tile_pool(name="wpool", bufs=1) as wpool,
            tc.tile_pool(name="apool", bufs=1) as apool,
            tc.tile_pool(name="cpool", bufs=1) as cpool,
            tc.tile_pool(name="xepool", bufs=2) as xepool,
            tc.tile_pool(name="tmppool", bufs=2) as tmppool,
            tc.tile_pool(name="ckpool", bufs=1) as ckpool,
            tc.tile_pool(name="xkpool", bufs=16) as xkpool,
            tc.tile_pool(name="accpool", bufs=1) as accpool,
            tc.tile_pool(name="ps", bufs=4, space="PSUM") as pspool,
        ):
            # ---- fp32 constants on the scalar queue (land ~2us) ----
            bias_sb = cpool.tile([P, D_OUT], F32, tag="badj")
            nc.scalar.dma_start(bias_sb[:], badj)
            svec_sb = cpool.tile([P, D_OUT], F32, tag="svec")
            nc.scalar.dma_start(svec_sb[:], svec)
            al_sb = cpool.tile([P, 8], F32, tag="al")
            nc.scalar.dma_start(al_sb[:], alpha)

            # ---- resident fp8 A / W: 4 quarter DMAs each (8KB runs) ----
            a_q = []
            w_q = []
            for q in range(4):
                ta = apool.tile([P, 4, 2, C], F8, tag=f"a{q}", name=f"a{q}")
                nc.sync.dma_start(ta[:], atR[:, q * 4 : (q + 1) * 4, :, :])
                a_q.append(ta)
                tw = wpool.tile([P, 4, 2, D_OUT], F8, tag=f"w{q}", name=f"w{q}")
                nc.sync.dma_start(tw[:], wR[:, q * 4 : (q + 1) * 4, :, :])
                w_q.append(tw)

            # ---- combine accumulator init (DVE idle early): exact rank-1
            # term acc[t, d] = alpha[t] * S[d]
            acc = accpool.tile([P, 4, 2, D_OUT], F32)
            for tb in range(4):
                for ti in range(2):
                    nc.vector.tensor_scalar(
                        acc[:, tb, ti, :],
                        svec_sb[:],
                        al_sb[:, tb * 2 + ti : tb * 2 + ti + 1],
                        None,
                        mybir.AluOpType.mult,
                    )

            ck = {}
            xk = {}
            _ck_blk0 = [0, 4, 8, 16, 24]  # chunk start offsets in cwtR blocks

            def load_ck(b):
                nsl = len(CHUNKS[b]) * 4
                t = ckpool.tile([P, nsl, 2, C], F8, tag=f"ck{b}", name=f"ck{b}")
                nc.gpsimd.dma_start(
                    t[:], cwtR[:, _ck_blk0[b] : _ck_blk0[b] + nsl, :, :]
                )
                ck[b] = t

            def load_xk(b):
                # split across the scalar and sync queues so the tiles land
                # in half the serial time after the AllGather posts
                if b < 2:
                    for u in range(4):
                        t = xkpool.tile(
                            [P, 2, D_OUT], F8, tag="xk", name=f"xk_{b}_{u}"
                        )
                        eng = nc.scalar if u % 2 == 0 else nc.sync
                        eng.dma_start(t[:, 0, :], xgA[b][:, 2 * u, :])
                        eng.dma_start(t[:, 1, :], xgA[b][:, 2 * u + 1, :])
                        xk[(b, u)] = t
                else:
                    for j in range(E):
                        t = xkpool.tile(
                            [P, 2, D_OUT], F8, tag="xk", name=f"xk_{b}_{j}"
                        )
                        eng = nc.scalar if j % 2 == 0 else nc.sync
                        eng.dma_start(t[:], xgC[b - 2][:, j, :, :])
                        xk[(b, j)] = t

            # ---------------- expert GEMM (fp8 DoubleRow) ----------------
            for ci in range(8):
                ps = pspool.tile([P, 2, 512], F32, tag="ps", name=f"ps_e{ci}")
                for kp in range(KP1):
                    lhsT = a_q[kp // 4][:, kp % 4, :, ci * 128 : (ci + 1) * 128]
                    for h in range(2):
                        nc.tensor.matmul(
                            ps[:, h, :],
                            lhsT,
                            w_q[kp // 4][:, kp % 4, :, h * 512 : (h + 1) * 512],
                            start=(kp == 0),
                            stop=(kp == KP1 - 1),
                            perf_mode=DR,
                        )
                # X'_ci = psum/64 + (bias - mu), cast fp8
                xe = xepool.tile([P, D_OUT], F8, tag="xe")
                for h in range(2):
                    sl = slice(h * 512, (h + 1) * 512)
                    tmp = tmppool.tile([P, 512], F32, tag="tmp")
                    nc.vector.tensor_scalar_mul(tmp[:], ps[:, h, :], 0.015625)
                    nc.vector.tensor_tensor(
                        xe[:, sl], tmp[:], bias_sb[:, sl], mybir.AluOpType.add
                    )
                b = next(i for i, ch in enumerate(CHUNKS) if ci in ch)
                off = ci - CHUNKS[b][0]
                nc.gpsimd.dma_start(
                    xh[b][:, off * D_OUT : (off + 1) * D_OUT], xe[:]
                )
                if ci == CHUNKS[b][-1]:
                    nc.gpsimd.collective_compute(
                        "AllGather",
                        mybir.AluOpType.bypass,
                        replica_groups=rg,
                        ins=[xh[b].ap().opt()],
                        outs=[xg[b].ap().opt()],
                    )
                    load_xk(b)  # scalar/sync queues, gated on the AG

            # ck loads after all evictions + AG triggers on the gpsimd
            # queue (one DMA per chunk; a stall here cannot delay triggers)
            for b in range(NCH):
                load_ck(b)

            # HAM keep-warm filler: bridge expert-GEMM end -> first xk
            # tiles so the combine starts on a warm (8/8) clock gate.
            for g in range(NFILL):
                psf = pspool.tile([P, 2, 512], F32, tag="ps", name=f"ps_f{g}")
                for i in range(8):
                    nc.tensor.matmul(
                        psf[:, i % 2, :],
                        a_q[0][:, 0, :, :128],
                        w_q[0][:, 0, :, (i % 2) * 512 : (i % 2) * 512 + 512],
                        start=(i < 2),
                        stop=(i >= 6),
                        perf_mode=DR,
                    )
                tmpf = tmppool.tile([P, 512], F32, tag="tmp")
                nc.vector.tensor_copy(tmpf[:], psf[:, 0, :])

            # ---------------- combine GEMM (fp8 DoubleRow) ----------------
            for b in range(NCH):
                slots = len(CHUNKS[b]) * 4
                for tb in range(4):
                    for ti in range(2):
                        pst = pspool.tile(
                            [P, 2, 512], F32, tag="ps", name=f"ps_c{b}_{tb}_{ti}"
                        )
                        for s in range(slots):
                            lhsT = ck[b][
                                :,
                                s,
                                :,
                                tb * 256 + ti * 128 : tb * 256 + (ti + 1) * 128,
                            ]
                            for h in range(2):
                                nc.tensor.matmul(
                                    pst[:, h, :],
                                    lhsT,
                                    xk[(b, s)][:, :, h * 512 : (h + 1) * 512],
                                    start=(s == 0),
                                    stop=(s == slots - 1),
                                    perf_mode=DR,
                                )
                        for h in range(2):
                            sl = slice(h * 512, (h + 1) * 512)
                            nc.vector.tensor_tensor(
                                acc[:, tb, ti, sl],
                                pst[:, h, :],
                                acc[:, tb, ti, sl],
                                mybir.AluOpType.add,
                            )
                            if b == NCH - 1:
                                # stream each half out as soon as it's final
                                nc.sync.dma_start(
                                    out4[:, tb, ti, sl], acc[:, tb, ti, sl]
                                )

    nc.compile()
    return nc


def _prep_inputs(expert_input, weight, bias, combine_weights):
    import ml_dtypes

    f8 = ml_dtypes.float8_e4m3
    f32 = np.float32

    def q8(x):
        return np.clip(x, -240.0, 240.0).astype(f8)

    def pmajor(x, n_tiles):
        # [n_tiles*128, F] row-major -> [128, n_tiles*F] partition-major
        F = x.shape[1]
        return np.ascontiguousarray(
            x.reshape(n_tiles, P, F).transpose(1, 0, 2).reshape(P, n_tiles * F)
        )

    A8 = [q8(expert_input[e]) for e in range(E)]  # [C, D_IN]
    W8 = [q8(64.0 * weight[e]) for e in range(E)]  # [D_IN, D_OUT]
    # exact colsum of the fp8 pipeline's X (colsum commutes with the GEMM)
    S = np.zeros(D_OUT, dtype=np.float64)
    for e in range(E):
        S += (
            A8[e].astype(np.float64).sum(0) @ W8[e].astype(np.float64)
        ) / 64.0 + C * bias[e].reshape(-1).astype(np.float64)
    mu = (S / N_TOK).astype(f32)
    Sf = S.astype(f32)

    order = _ck_ktile_order()

    in_maps = []
    for e in range(E):
        cw = combine_weights[e * C : (e + 1) * C, :]
        r = cw.astype(np.float64).sum(1)
        cwt = q8(cw - 0.5).T  # [8192 (k), 1024 (own tokens)]
        cwt_tiles = np.ascontiguousarray(cwt).reshape(64, P, C)[order]
        in_maps.append(
            {
                "at2": pmajor(np.ascontiguousarray(A8[e].T), D_IN // P),
                "w2": pmajor(np.ascontiguousarray(W8[e]), D_IN // P),
                "badj": np.ascontiguousarray(
                    np.broadcast_to(
                        (bias[e].reshape(1, D_OUT) - mu[None, :]).astype(f32),
                        (P, D_OUT),
                    )
                ),
                "svec": np.ascontiguousarray(
                    np.broadcast_to(Sf[None, :], (P, D_OUT)).astype(f32)
                ),
                "alpha": np.ascontiguousarray(
                    (r / N_TOK).astype(f32).reshape(8, P).T
                ),
                "cwt2": np.ascontiguousarray(
                    cwt_tiles.transpose(1, 0, 2).reshape(P, 64 * C)
                ),
            }
        )
    return in_maps


def _run(expert_input, weight, bias, combine_weights, trace=False):
    from concourse import bass_utils

    global _cached
    if _cached is None:
        _cached = _build()
    nc = _cached
    in_maps = _prep_inputs(expert_input, weight, bias, combine_weights)
    r = bass_utils.run_bass_kernel_spmd(
        nc, in_maps, core_ids=list(range(E)), trace=trace
    )
    # out is [128, 8, 1024] p-major: token t = (tb*2+ti)*128 + p
    blocks = [
        r.results[e]["out"]
        .reshape(P, 8, D_OUT)
        .transpose(1, 0, 2)
        .reshape(C, D_OUT)
        for e in range(E)
    ]
    output = np.concatenate(blocks, axis=0)
    return output.astype(np.float32, copy=False), r


def kernel(expert_input, weight, bias, combine_weights):
    output, _ = _run(expert_input, weight, bias, combine_weights)
    return output


# revision 15
# speedup vs baseline: 1.0521x; 1.0128x over previous
"""Expert-parallel MoE block (dense path) on 8 Trainium2 NeuronCores.

Reference computation (E=8, C=1024, D_IN=4096, D_OUT=1024, N_TOK=8192):
    expert_out = einsum('eci,eio->eco', expert_input, weight) + bias   # [E,C,D_OUT]
    output     = combine_weights @ expert_out.reshape(E*C, D_OUT)      # [N_TOK,D_OUT]

Sharding (expert-parallel):
  Core e holds expert e: computes X_e = expert_input[e] @ weight[e] + bias[e]
  ([C, D_OUT]); on-device AllGathers assemble X; core e computes its token
  slice of the combine; the host re-assembles the 8 row blocks.

Numerics: all matmuls are fp8-e4m3 in DoubleRow mode (2 k-tiles per pass).
The error budget works out because the output is dominated by the bias/mean
structure, which is carried exactly in fp32:
  - Expert GEMM: A in fp8, W*64 in fp8 (scaling keeps W out of e4m3
    subnormals); PSUM fp32; drain computes X' = psum/64 + (bias - mu) on DVE
    and casts to fp8.  mu = colmean(X) is computed EXACTLY on the host from
    the quantized inputs (colsum commutes with the GEMM).
  - Combine: out = CW'@X' + rowsum(CW) (x) colsum(X) / K, with CW' = CW-0.5
    quantized fp8 on the host. Centering both operands halves their rms so
    the fp8 noise lands ~1e-3 relative, and the exact rank-1 correction is a
    per-(tb,ti) DVE tensor_scalar init of the fp32 accumulator.
Measured end-to-end rel err ~1.6e-3.

Performance structure (v4): the PE is issue-limited at the (GPIO-throttled,
13/16 = 1.95GHz) clock; the kernel keeps it on real work end to end.
  - ALL large tensors are passed from the host in partition-major layout
    ([128, bytes-per-partition] with each partition's data contiguous), so
    every DMA moves 2-16KB contiguous runs instead of the 1KB rows of the
    natural layouts (which measured ~140GB/s and starved the expert GEMM).
    A and W load as 4 quarter DMAs each; each ck chunk is ONE DMA.
  - Expert GEMM sweeps c-tiles; X' c-tiles evict to p-major xh buffers and
    AllGather in chunks [1,1,2,2,2]: the 1-tile chunks trigger the (fixed,
    ~60-80us) first-collective barrier as early as every rank can, and the
    later 2-tile chunks amortize the ~2us inter-collective gap.
  - The combine consumes chunks in arrival order with a SBUF fp32
    accumulator; a short HAM keep-warm filler bridges expert-end -> first
    xk tiles so the clock gate never drops to 4/8.
"""

import numpy as np

E = 8
C = 1024
D_IN = 4096
D_OUT = 1024
N_TOK = E * C
P = 128

KP1 = D_IN // (2 * P)  # 16 k-tile pairs in the expert GEMM
CHUNKS = [[0], [1], [2, 3], [4, 5], [6, 7]]
NCH = len(CHUNKS)
NFILL = 8  # HAM keep-warm groups between expert GEMM and combine

# cwt2 block order: for the cross-rank-paired 1-tile chunks, tile u pairs
# k-tiles (2u*8+b, (2u+1)*8+b); for 2-tile chunks, tile j pairs (j*8+c0,
# j*8+c0+1).  The host lays the 64 k-tiles out in exactly this order so
# each chunk's ck tiles are one contiguous DMA.
def _ck_ktile_order():
    order = []
    for b in (0, 1):
        for u in range(4):
            order += [(2 * u) * 8 + b, (2 * u + 1) * 8 + b]
    for b in (2, 3, 4):
        c0 = CHUNKS[b][0]
        for j in range(E):
            order += [j * 8 + c0, j * 8 + c0 + 1]
    return order


_cached = None


def _build():
    import concourse.bass as bass  # noqa: F401
    import concourse.mybir as mybir
    import concourse.tile as tile
    from concourse import bacc

    F8 = mybir.dt.float8e4
    F32 = mybir.dt.float32
    DR = mybir.MatmulPerfMode.DoubleRow

    nc = bacc.Bacc("TRN2", target_bir_lowering=False, debug=False, num_devices=E)

    # partition-major inputs: [128, contiguous bytes per partition]
    at2 = nc.dram_tensor("at2", [P, KP1 * 2 * C], F8, kind="ExternalInput").ap()
    w2 = nc.dram_tensor("w2", [P, KP1 * 2 * D_OUT], F8, kind="ExternalInput").ap()
    badj = nc.dram_tensor("badj", [P, D_OUT], F32, kind="ExternalInput").ap()
    svec = nc.dram_tensor("svec", [P, D_OUT], F32, kind="ExternalInput").ap()
    alpha = nc.dram_tensor("alpha", [P, 8], F32, kind="ExternalInput").ap()
    cwt2 = nc.dram_tensor("cwt2", [P, 64 * C], F8, kind="ExternalInput").ap()
    out = nc.dram_tensor("out", [P, 8 * D_OUT], F32, kind="ExternalOutput").ap()

    # AllGather staging, p-major: xh[b] rank-local [128, len*1024]; the AG
    # concatenates rank blocks so xg[b] rows [j*128:(j+1)*128] are rank j.
    xh = [
        nc.dram_tensor(f"xh{b}", [P, len(ch) * D_OUT], F8)
        for b, ch in enumerate(CHUNKS)
    ]
    xg = [
        nc.dram_tensor(
            f"xg{b}", [E * P, len(ch) * D_OUT], F8, addr_space="Shared"
        )
        for b, ch in enumerate(CHUNKS)
    ]

    atR = at2.rearrange("p (kp t c) -> p kp t c", t=2, c=C)  # [128,16,2,1024]
    wR = w2.rearrange("p (kp t d) -> p kp t d", t=2, d=D_OUT)
    cwtR = cwt2.rearrange("p (blk t c) -> p blk t c", t=2, c=C)  # [128,32,2,1024]
    xgA = [xg[b].rearrange("(j p) c -> p j c", p=P) for b in range(2)]
    xgC = [
        xg[b].rearrange("(j p) (t c) -> p j t c", p=P, t=2) for b in range(2, NCH)
    ]
    out4 = out.rearrange("p (tb ti d) -> p tb ti d", ti=2, d=D_OUT)

    rg = [list(range(E))]

    with tile.TileContext(nc) as tc:
        with (
            tc.tile_pool(name="wpool", bufs=1) as wpool,
            tc.tile_pool(name="apool", bufs=1) as apool,
            tc.tile_pool(name="cpool", bufs=1) as cpool,
            tc.tile_pool(name="xepool", bufs=2) as xepool,
            tc.tile_pool(name="tmppool", bufs=2) as tmppool,
            tc.tile_pool(name="ckpool", bufs=1) as ckpool,
            tc.tile_pool(name="xkpool", bufs=16) as xkpool,
            tc.tile_pool(name="accpool", bufs=1) as accpool,
            tc.tile_pool(name="ps", bufs=4, space="PSUM") as pspool,
        ):
            # ---- fp32 constants on the vector queue (land early, off the
            # A/W feed queues) ----
            bias_sb = cpool.tile([P, D_OUT], F32, tag="badj")
            nc.gpsimd.dma_start(bias_sb[:], badj)
            svec_sb = cpool.tile([P, D_OUT], F32, tag="svec")
            nc.gpsimd.dma_start(svec_sb[:], svec)
            al_sb = cpool.tile([P, 8], F32, tag="al")
            nc.gpsimd.dma_start(al_sb[:], alpha)

            # ---- resident fp8 A / W: 4 quarter DMAs each (8KB runs), A on
            # the sync queue and W on the scalar queue — one hardware-DGE
            # queue sustains only ~140GB/s, two run concurrently ----
            a_q = []
            w_q = []
            for q in range(4):
                ta = apool.tile([P, 4, 2, C], F8, tag=f"a{q}", name=f"a{q}")
                nc.sync.dma_start(ta[:], atR[:, q * 4 : (q + 1) * 4, :, :])
                a_q.append(ta)
                tw = wpool.tile([P, 4, 2, D_OUT], F8, tag=f"w{q}", name=f"w{q}")
                nc.scalar.dma_start(tw[:], wR[:, q * 4 : (q + 1) * 4, :, :])
                w_q.append(tw)

            # ---- combine accumulator init (DVE idle early): exact rank-1
            # term acc[t, d] = alpha[t] * S[d]
            acc = accpool.tile([P, 4, 2, D_OUT], F32)
            for tb in range(4):
                for ti in range(2):
                    nc.vector.tensor_scalar(
                        acc[:, tb, ti, :],
                        svec_sb[:],
                        al_sb[:, tb * 2 + ti : tb * 2 + ti + 1],
                        None,
                        mybir.AluOpType.mult,
                    )

            ck = {}
            xk = {}
            _ck_blk0 = [0, 4, 8, 16, 24]  # chunk start offsets in cwtR blocks

            def load_ck(b):
                nsl = len(CHUNKS[b]) * 4
                t = ckpool.tile([P, nsl, 2, C], F8, tag=f"ck{b}", name=f"ck{b}")
                nc.gpsimd.dma_start(
                    t[:], cwtR[:, _ck_blk0[b] : _ck_blk0[b] + nsl, :, :]
                )
                ck[b] = t

            def load_xk(b):
                # split across the scalar and sync queues so the tiles land
                # in half the serial time after the AllGather posts
                if b < 2:
                    for u in range(4):
                        t = xkpool.tile(
                            [P, 2, D_OUT], F8, tag="xk", name=f"xk_{b}_{u}"
                        )
                        eng = nc.scalar if u % 2 == 0 else nc.sync
                        eng.dma_start(t[:, 0, :], xgA[b][:, 2 * u, :])
                        eng.dma_start(t[:, 1, :], xgA[b][:, 2 * u + 1, :])
                        xk[(b, u)] = t
                else:
                    for j in range(E):
                        t = xkpool.tile(
                            [P, 2, D_OUT], F8, tag="xk", name=f"xk_{b}_{j}"
                        )
                        eng = nc.scalar if j % 2 == 0 else nc.sync
                        eng.dma_start(t[:], xgC[b - 2][:, j, :, :])
                        xk[(b, j)] = t

            # ---------------- expert GEMM (fp8 DoubleRow) ----------------
            # kp-outer over c-tile HALVES (4 c-tiles x 2 psum halves = all
            # 8 PSUM banks): the PE consumes A/W quarters in arrival order
            # at DMA pace with no stall-then-catchup, and c-tiles 0-3
            # complete together right after the last quarter lands.
            for half in range(2):
                pss = [
                    pspool.tile([P, 2, 512], F32, tag="ps", name=f"ps_e{half}_{c}")
                    for c in range(4)
                ]
                for kp in range(KP1):
                    for c in range(4):
                        ci = half * 4 + c
                        lhsT = a_q[kp // 4][
                            :, kp % 4, :, ci * 128 : (ci + 1) * 128
                        ]
                        for h in range(2):
                            nc.tensor.matmul(
                                pss[c][:, h, :],
                                lhsT,
                                w_q[kp // 4][
                                    :, kp % 4, :, h * 512 : (h + 1) * 512
                                ],
                                start=(kp == 0),
                                stop=(kp == KP1 - 1),
                                perf_mode=DR,
                            )
                for c in range(4):
                    ci = half * 4 + c
                    # X'_ci = psum/64 + (bias - mu), cast fp8
                    xe = xepool.tile([P, D_OUT], F8, tag="xe")
                    for h in range(2):
                        sl = slice(h * 512, (h + 1) * 512)
                        tmp = tmppool.tile([P, 512], F32, tag="tmp")
                        nc.vector.tensor_scalar_mul(
                            tmp[:], pss[c][:, h, :], 0.015625
                        )
                        nc.vector.tensor_tensor(
                            xe[:, sl], tmp[:], bias_sb[:, sl], mybir.AluOpType.add
                        )
                    b = next(i for i, ch in enumerate(CHUNKS) if ci in ch)
                    off = ci - CHUNKS[b][0]
                    nc.gpsimd.dma_start(
                        xh[b][:, off * D_OUT : (off + 1) * D_OUT], xe[:]
                    )
                    if ci == CHUNKS[b][-1]:
                        nc.gpsimd.collective_compute(
                            "AllGather",
                            mybir.AluOpType.bypass,
                            replica_groups=rg,
                            ins=[xh[b].ap().opt()],
                            outs=[xg[b].ap().opt()],
                        )
                        load_xk(b)  # scalar/sync queues, gated on the AG

            # ck loads after all evictions + AG triggers on the gpsimd
            # queue (one DMA per chunk; a stall here cannot delay triggers)
            for b in range(NCH):
                load_ck(b)

            # HAM keep-warm filler: bridge expert-GEMM end -> first xk
            # tiles so the combine starts on a warm (8/8) clock gate.
            for g in range(NFILL):
                psf = pspool.tile([P, 2, 512], F32, tag="ps", name=f"ps_f{g}")
                for i in range(8):
                    nc.tensor.matmul(
                        psf[:, i % 2, :],
                        a_q[0][:, 0, :, :128],
                        w_q[0][:, 0, :, (i % 2) * 512 : (i % 2) * 512 + 512],
                        start=(i < 2),
                        stop=(i >= 6),
                        perf_mode=DR,
                    )
                tmpf = tmppool.tile([P, 512], F32, tag="tmp")
                nc.vector.tensor_copy(tmpf[:], psf[:, 0, :])

            # ---------------- combine GEMM (fp8 DoubleRow) ----------------
            for b in range(NCH):
                slots = len(CHUNKS[b]) * 4
                for tb in range(4):
                    for ti in range(2):
                        pst = pspool.tile(
                            [P, 2, 512], F32, tag="ps", name=f"ps_c{b}_{tb}_{ti}"
                        )
                        for s in range(slots):
                            lhsT = ck[b][
                                :,
                                s,
                                :,
                                tb * 256 + ti * 128 : tb * 256 + (ti + 1) * 128,
                            ]
                            for h in range(2):
                                nc.tensor.matmul(
                                    pst[:, h, :],
                                    lhsT,
                                    xk[(b, s)][:, :, h * 512 : (h + 1) * 512],
                                    start=(s == 0),
                                    stop=(s == slots - 1),
                                    perf_mode=DR,
                                )
                        for h in range(2):
                            sl = slice(h * 512, (h + 1) * 512)
                            nc.vector.tensor_tensor(
                                acc[:, tb, ti, sl],
                                pst[:, h, :],
                                acc[:, tb, ti, sl],
                                mybir.AluOpType.add,
                            )
                            if b == NCH - 1:
                                # stream each half out as soon as it's final
                                nc.sync.dma_start(
                                    out4[:, tb, ti, sl], acc[:, tb, ti, sl]
                                )

    nc.compile()
    return nc


def _prep_inputs(expert_input, weight, bias, combine_weights):
    import ml_dtypes

    f8 = ml_dtypes.float8_e4m3
    f32 = np.float32

    def q8(x):
        return np.clip(x, -240.0, 240.0).astype(f8)

    def pmajor(x, n_tiles):
        # [n_tiles*128, F] row-major -> [128, n_tiles*F] partition-major
        F = x.shape[1]
        return np.ascontiguousarray(
            x.reshape(n_tiles, P, F).transpose(1, 0, 2).reshape(P, n_tiles * F)
        )

    A8 = [q8(expert_input[e]) for e in range(E)]  # [C, D_IN]
    W8 = [q8(64.0 * weight[e]) for e in range(E)]  # [D_IN, D_OUT]
    # exact colsum of the fp8 pipeline's X (colsum commutes with the GEMM)
    S = np.zeros(D_OUT, dtype=np.float64)
    for e in range(E):
        S += (
            A8[e].astype(np.float64).sum(0) @ W8[e].astype(np.float64)
        ) / 64.0 + C * bias[e].reshape(-1).astype(np.float64)
    mu = (S / N_TOK).astype(f32)
    Sf = S.astype(f32)

    order = _ck_ktile_order()

    in_maps = []
    for e in range(E):
        cw = combine_weights[e * C : (e + 1) * C, :]
        r = cw.astype(np.float64).sum(1)
        cwt = q8(cw - 0.5).T  # [8192 (k), 1024 (own tokens)]
        cwt_tiles = np.ascontiguousarray(cwt).reshape(64, P, C)[order]
        in_maps.append(
            {
                "at2": pmajor(np.ascontiguousarray(A8[e].T), D_IN // P),
                "w2": pmajor(np.ascontiguousarray(W8[e]), D_IN // P),
                "badj": np.ascontiguousarray(
                    np.broadcast_to(
                        (bias[e].reshape(1, D_OUT) - mu[None, :]).astype(f32),
                        (P, D_OUT),
                    )
                ),
                "svec": np.ascontiguousarray(
                    np.broadcast_to(Sf[None, :], (P, D_OUT)).astype(f32)
                ),
                "alpha": np.ascontiguousarray(
                    (r / N_TOK).astype(f32).reshape(8, P).T
                ),
                "cwt2": np.ascontiguousarray(
                    cwt_tiles.transpose(1, 0, 2).reshape(P, 64 * C)
                ),
            }
        )
    return in_maps


def _run(expert_input, weight, bias, combine_weights, trace=False):
    from concourse import bass_utils

    global _cached
    if _cached is None:
        _cached = _build()
    nc = _cached
    in_maps = _prep_inputs(expert_input, weight, bias, combine_weights)
    r = bass_utils.run_bass_kernel_spmd(
        nc, in_maps, core_ids=list(range(E)), trace=trace
    )
    # out is [128, 8, 1024] p-major: token t = (tb*2+ti)*128 + p
    blocks = [
        r.results[e]["out"]
        .reshape(P, 8, D_OUT)
        .transpose(1, 0, 2)
        .reshape(C, D_OUT)
        for e in range(E)
    ]
    output = np.concatenate(blocks, axis=0)
    return output.astype(np.float32, copy=False), r


def kernel(expert_input, weight, bias, combine_weights):
    output, _ = _run(expert_input, weight, bias, combine_weights)
    return output
